# revision 54
# baseline (speedup 1.0000x reference)
"""Causal self-attention (B=2,S=2048,C=768,H=12,D=64) on 8 trn2 NeuronCores.

Sharding: core c -> batch b=c//4, head-group g=c%4 (3 heads each).
Per-core program (SPMD, same NEFF, different data):
  A: qk^T [feat, s]  = Wqk^T x^T          (feat-major, matmul-native layout)
  B: V    [t, d]     = x W_v  (+ ones col -> softmax denominators for free)
  C: per (s-block, head): scores^T tiles [t,s] -> exp (ACT, scale=1/8,
     causal via suffix-trim + gpsimd triangular select) ->
     PV: out^T[d,s] + denominator row, accumulated over t on PE ->
     normalize on device: r=1/denom (DVE), gpsimd partition_broadcast,
     out_norm = out^T * r (DVE)
  D: proj-T summed over heads: y^T[c,s] = sum_h Wp_h^T outn_h^T
Host: y[b] = sum_g y_g^T.T + b_proj.
"""

import numpy as np
import ml_dtypes

S = 2048
C = 768
D = 64
HL = 3  # heads per core
KC = C // 128  # 6 contraction tiles
NB = S // 512  # 4 s-blocks
TT = S // 128  # 16 t-tiles
SB = 512

# feature placement inside qk_sb [128, 3, S]: host packs W cols as
# [q0 q1 | k0 k1 | q2 k2], so heads 0/1 have q,k natively at matching
# partition offsets (matmul needs equal base partitions); only k2 gets a
# small DMA replica aligned with q2.
QPOS = [(0, 0), (0, 64), (2, 0)]

_NC_CACHE = {}


def _build_nc():
    import concourse.bass as bass
    import concourse.tile as tile
    import concourse.mybir as mybir
    from concourse import bacc

    bf16 = mybir.dt.bfloat16
    f32 = mybir.dt.float32
    AF = mybir.ActivationFunctionType

    nc = bacc.Bacc(
        "TRN2",
        target_bir_lowering=False,
        debug=False,
        enable_asserts=False,
        num_devices=8,
    )
    xT = nc.declare_dram_parameter("xT", [C, S], bf16, isOutput=False)
    wqk = nc.declare_dram_parameter("wqk", [C, 2 * HL * D], bf16, isOutput=False)
    wv = nc.declare_dram_parameter("wv", [C, HL * D], bf16, isOutput=False)
    wp01 = nc.declare_dram_parameter("wp01", [128, C], bf16, isOutput=False)
    wp2 = nc.declare_dram_parameter("wp2", [D, C], bf16, isOutput=False)
    bqk = nc.declare_dram_parameter("bqk", [2 * HL * D], f32, isOutput=False)
    bv = nc.declare_dram_parameter("bv", [HL * D], f32, isOutput=False)
    yT = nc.declare_dram_parameter("yT", [C, S], bf16, isOutput=True)

    with tile.TileContext(nc) as tc:
        with (
            tc.tile_pool(name="const", bufs=1) as cpool,
            tc.tile_pool(name="exp", bufs=5) as epool,
            tc.tile_pool(name="ou", bufs=4) as oupool,
            tc.tile_pool(name="rr", bufs=4) as rpool,
            tc.tile_pool(name="ysb", bufs=6) as ypool,
            tc.tile_pool(name="ps1", bufs=2, space="PSUM") as ps1,
            tc.tile_pool(name="psC", bufs=4, space="PSUM") as psC,
            tc.tile_pool(name="psPV", bufs=2, space="PSUM") as psPV,
        ):
            # ---- persistent SBUF tensors ----
            # weights first (phase A needs them immediately); x in two halves
            # so the first A groups start after ~1.5MB instead of 3MB
            wqk_sb = cpool.tile([128, KC, 2 * HL * D], bf16)
            nc.sync.dma_start(wqk_sb, wqk.rearrange("(o p) n -> p o n", p=128))
            bqk_sb = cpool.tile([128, 3], f32)
            nc.sync.dma_start(bqk_sb, bqk.rearrange("(o p) -> p o", p=128))
            xt_sb = cpool.tile([128, KC, S], bf16)
            xTr = xT.rearrange("(o p) s -> p o s", p=128)
            for j in range(NB):
                nc.sync.dma_start(
                    xt_sb[:, :, j * SB : (j + 1) * SB],
                    xTr[:, :, j * SB : (j + 1) * SB],
                )
            wv_sb = cpool.tile([128, KC, HL * D], bf16)
            nc.sync.dma_start(wv_sb, wv.rearrange("(o p) n -> p o n", p=128))
            wp01_sb = cpool.tile([128, C], bf16)
            nc.sync.dma_start(wp01_sb, wp01[:])
            wp2_sb = cpool.tile([D, C], bf16)
            nc.sync.dma_start(wp2_sb, wp2[:])
            bv_sb = cpool.tile([128, HL * D], f32)
            nc.sync.dma_start(bv_sb, bv[None, :].to_broadcast((128, HL * D)))
            qk_sb = cpool.tile([128, 3, S], bf16)
            qk2_sb = cpool.tile([D, S], bf16)  # k2 replica aligned with q2
            v1_sb = cpool.tile([128, TT, HL, D + 1], bf16)
            nc.gpsimd.memset(v1_sb, 1.0)  # ones col; d cols overwritten below

            # ---- phase A: qk^T = Wqk^T @ x^T  -> [feat, s] ----
            def phase_a(nf):
                for j in range(NB):
                    ps = ps1.tile([128, SB], f32, tag="p1", name=f"psa{nf}_{j}")
                    for kc in range(KC):
                        nc.tensor.matmul(
                            ps,
                            wqk_sb[:, kc, nf * 128 : (nf + 1) * 128],
                            xt_sb[:, kc, j * SB : (j + 1) * SB],
                            start=(kc == 0),
                            stop=(kc == KC - 1),
                        )
                    nc.vector.tensor_add(
                        qk_sb[:, nf, j * SB : (j + 1) * SB],
                        ps,
                        bqk_sb[:, nf : nf + 1].to_broadcast((128, SB)),
                    )

            # ---- phase B (emitted later, after j0 scores, for ACT overlap) ----
            def phase_b():
                for tt in range(TT):
                    ps = ps1.tile([128, SB], f32, tag="p1", name=f"psb{tt}")
                    for kc in range(KC):
                        nc.tensor.matmul(
                            ps[:, 0 : HL * D],
                            xt_sb[:, kc, tt * 128 : (tt + 1) * 128],
                            wv_sb[:, kc, :],
                            start=(kc == 0),
                            stop=(kc == KC - 1),
                        )
                    nc.vector.tensor_add(
                        v1_sb[:, tt, :, 0:D],
                        ps[:, 0 : HL * D].rearrange("p (h d) -> p h d", h=HL),
                        bv_sb.rearrange("p (h d) -> p h d", h=HL),
                    )

            # ---- phases C (scores^T + exp + PV + normalize) / D (proj) ----
            def scores_pair(j, h, p, ex):
                """One pair of scores^T t-tiles -> exp, for head h, s-block j."""
                nf, po = QPOS[h]
                if True:
                    for k in range(2):
                        tt = 2 * p + k
                        i = tt - 4 * j
                        lo = 128 * i if i >= 0 else 0
                        ps = psC.tile([128, SB], f32, tag="c", name=f"c{j}_{h}_{p}_{k}")
                        kT = (
                            qk2_sb[0:D, tt * 128 : (tt + 1) * 128]
                            if h == 2
                            else qk_sb[po : po + D, 1, tt * 128 : (tt + 1) * 128]
                        )
                        nc.tensor.matmul(
                            ps[:, lo:SB],
                            kT,
                            qk_sb[po : po + D, nf, j * SB + lo : (j + 1) * SB],
                            start=True,
                            stop=True,
                        )
                        nc.scalar.activation(
                            ex[:, tt * SB + lo : (tt + 1) * SB],
                            ps[:, lo:SB],
                            AF.Exp,
                            scale=0.125,
                        )
                    for k in range(2):
                        tt = 2 * p + k
                        i = tt - 4 * j
                        if i >= 0:
                            lo = 128 * i
                            # causal triangle: keep t_local <= s_rel, else 0
                            st = tt * SB + lo
                            nc.gpsimd.affine_select(
                                out=ex[:, st : st + 128],
                                in_=ex[:, st : st + 128],
                                compare_op=mybir.AluOpType.is_ge,
                                fill=0.0,
                                base=0,
                                pattern=[[1, 128]],
                                channel_multiplier=-1,
                            )

            def pv_start(j, h):
                return psPV.tile([D + 1, SB], f32, tag="pv", name=f"po{j}_{h}")

            def pv_pair(j, h, p, po, ex, npair):
                for k in range(2):
                    tt = 2 * p + k
                    i = tt - 4 * j
                    lo = 128 * i if i >= 0 else 0
                    nc.tensor.matmul(
                        po[:, lo:SB],
                        v1_sb[:, tt, h, :],
                        ex[:, tt * SB + lo : (tt + 1) * SB],
                        start=(tt == 0),
                        stop=(tt == 2 * npair - 1),
                    )

            def pv_norm(j, h, po, dst):
                r = rpool.tile([1, SB], f32, tag="r", name=f"r{j}_{h}")
                nc.vector.reciprocal(r, po[D : D + 1, :])
                rb = rpool.tile([D, SB], f32, tag="rb", name=f"rb{j}_{h}")
                nc.gpsimd.partition_broadcast(rb, r)
                nc.vector.tensor_mul(dst, po[0:D, :], rb)

            def pv_head(j, h, ex, dst):
                npair = 2 * (j + 1)
                po = pv_start(j, h)
                for p in range(npair):
                    pv_pair(j, h, p, po, ex, npair)
                pv_norm(j, h, po, dst)

            def proj_j(j, comb, oun2):
                # h0+h1 stacked on partitions -> one K=128 MM, then h2 K=64
                for ct in range(KC):
                    py = ps1.tile([128, SB], f32, tag="p1", name=f"py{j}_{ct}")
                    nc.tensor.matmul(
                        py,
                        wp01_sb[:, ct * 128 : (ct + 1) * 128],
                        comb,
                        start=True,
                        stop=False,
                    )
                    nc.tensor.matmul(
                        py,
                        wp2_sb[:, ct * 128 : (ct + 1) * 128],
                        oun2,
                        start=False,
                        stop=True,
                    )
                    ys = ypool.tile([128, SB], bf16, tag="ysb", name=f"ys{j}_{ct}")
                    nc.vector.tensor_copy(ys, py)
                    nc.sync.dma_start(
                        yT[ct * 128 : (ct + 1) * 128, j * SB : (j + 1) * SB], ys
                    )

            # j=0 staged against phases A/B so ACT starts early
            ex_j0 = [
                epool.tile([128, TT * SB], bf16, tag="exp", name=f"exj0_{hh}")
                for hh in range(3)
            ]
            phase_a(0)
            phase_a(1)
            for p in range(2):
                scores_pair(0, 0, p, ex_j0[0])
                scores_pair(0, 1, p, ex_j0[1])
            phase_a(2)
            nc.sync.dma_start(qk2_sb[0:D, :], qk_sb[D:128, 2, :])  # k2 replica
            phase_b()  # PE does V while ACT chews j0 exps
            comb0 = oupool.tile([128, SB], bf16, tag="ou01", name="comb0")
            tmp0 = oupool.tile([D, SB], bf16, tag="outmp", name="tmp0")
            pv_head(0, 0, ex_j0[0], comb0[0:D, :])
            pv_head(0, 1, ex_j0[1], tmp0)
            nc.sync.dma_start(comb0[D:128, :], tmp0)
            for p in range(2):
                scores_pair(0, 2, p, ex_j0[2])
            o2_0 = oupool.tile([D, SB], bf16, tag="ou2", name="o2_0")
            pv_head(0, 2, ex_j0[2], o2_0)
            # prologue of the 1-block-lookahead pipeline: j=1 h0/h1 scores
            # (interleaved: h0 on PE rows 0-63, h1 on rows 64-127) keep ACT
            # fed while PE runs j0's projection.
            ex01 = [
                epool.tile([128, TT * SB], bf16, tag="exp", name=f"exp1_{hh}")
                for hh in range(2)
            ]
            for p in range(4):
                scores_pair(1, 0, p, ex01[0])
                scores_pair(1, 1, p, ex01[1])
            proj_j(0, comb0, o2_0)

            for j in range(1, NB):
                # steady state: j's h0/h1 exps are already queued; every PE
                # block (PV/proj) is followed by freshly queued score work so
                # the 4-deep psC backlog keeps ACT busy throughout.
                npair = 2 * (j + 1)
                comb = oupool.tile([128, SB], bf16, tag="ou01", name=f"comb{j}")
                tmp = oupool.tile([D, SB], bf16, tag="outmp", name=f"tmp{j}")
                pv_head(j, 0, ex01[0], comb[0:D, :])
                ex2 = epool.tile([128, TT * SB], bf16, tag="exp", name=f"ex2_{j}")
                for p in range(npair):
                    scores_pair(j, 2, p, ex2)
                pv_head(j, 1, ex01[1], tmp)
                nc.sync.dma_start(comb[D:128, :], tmp)
                if j < NB - 1:
                    ex01 = [
                        epool.tile(
                            [128, TT * SB], bf16, tag="exp", name=f"exn{j}_{hh}"
                        )
                        for hh in range(2)
                    ]
                    for p in range(2 * (j + 2)):
                        scores_pair(j + 1, 0, p, ex01[0])
                        scores_pair(j + 1, 1, p, ex01[1])
                o2 = oupool.tile([D, SB], bf16, tag="ou2", name=f"o2_{j}")
                pv_head(j, 2, ex2, o2)
                proj_j(j, comb, o2)
    nc.finalize()
    return nc


def _get_nc():
    if "nc" not in _NC_CACHE:
        _NC_CACHE["nc"] = _build_nc()
    return _NC_CACHE["nc"]


def kernel(x, W_attn, b_attn, W_proj, b_proj):
    from concourse.bass_utils import run_bass_kernel_spmd

    x = np.asarray(x, np.float32)
    W_attn = np.asarray(W_attn, np.float32)
    b_attn = np.asarray(b_attn, np.float32)
    W_proj = np.asarray(W_proj, np.float32)
    b_proj = np.asarray(b_proj, np.float32)
    bf = ml_dtypes.bfloat16

    nc = _get_nc()
    in_maps = []
    for c in range(8):
        b, g = c // 4, c % 4
        cs = slice(192 * g, 192 * (g + 1))
        Wq = W_attn[:, 0 * C : 1 * C][:, cs]
        Wk = W_attn[:, 1 * C : 2 * C][:, cs]
        Wv = W_attn[:, 2 * C : 3 * C][:, cs]
        in_maps.append(
            {
                "xT": np.ascontiguousarray(x[b].T).astype(bf),
                "wqk": np.ascontiguousarray(
                    np.concatenate(
                        [Wq[:, 0:128], Wk[:, 0:128], Wq[:, 128:192], Wk[:, 128:192]],
                        1,
                    )
                ).astype(bf),
                "wv": np.ascontiguousarray(Wv).astype(bf),
                "wp01": np.ascontiguousarray(W_proj[cs, :][0:128]).astype(bf),
                "wp2": np.ascontiguousarray(W_proj[cs, :][128:192]).astype(bf),
                "bqk": np.ascontiguousarray(
                    np.concatenate(
                        [
                            b_attn[0:C][cs][0:128],
                            b_attn[C : 2 * C][cs][0:128],
                            b_attn[0:C][cs][128:192],
                            b_attn[C : 2 * C][cs][128:192],
                        ]
                    )
                ).astype(np.float32),
                "bv": np.ascontiguousarray(b_attn[2 * C : 3 * C][cs]).astype(
                    np.float32
                ),
            }
        )

    res = run_bass_kernel_spmd(nc, in_maps, list(range(8)))
    _NC_CACHE["last_result"] = res

    out = np.zeros((2, S, C), np.float32)
    for c in range(8):
        b = c // 4
        yTc = np.asarray(res.results[c]["yT"], dtype=np.float32)  # [C, S]
        out[b] += yTc.T
    out += b_proj[None, None, :]
    return out
